# revision 30
# baseline (speedup 1.0000x reference)
"""Trainium2 Bass kernel for nn_Encoder_9818295238798.

Single-layer transformer encoder: embed -> QKV -> full softmax attention
-> 2-layer FFN (relu).  B=32, S=2048, D=64, VOCAB=10000.

Sharding: data-parallel over the batch dim, 4 sequences per core x 8 cores,
parameters replicated.

Layout strategy: everything on-chip lives transposed ([feature, token]) so
that every bias is a per-partition scalar and softmax reductions land in the
matmul contraction:
  - embedding rows are gathered pre-transposed via dma_gather(transpose=True)
    from a host-prepared bf16 emb padded to 128 cols (col 64 = 1.0 so the
    "ones row" used for the Q/K bias-fold comes along for free)
  - Q^T/K^T computed with the weight (with bias row appended) stationary,
    duplicated into partitions 0-63 / 64-127 via PE col-tiling so the scores
    matmuls can run 2-way row-packed (K=64 only half-fills the PE array)
  - scores computed transposed [t, s]; exp on ScalarE straight out of PSUM
    with the 1/sqrt(D) scale folded into the activation's free scale field
    (scores are bounded ~+-3 so no max-subtraction is needed)
  - softmax denominator rides along as a 65th (ones) column of V
  - upd normalization via a partition-broadcast of the reciprocal; V bias is
    folded into b1 on the host (rows of probs sum to 1)
  - FFN biases ride the contraction as appended ones rows; final transpose
    back on the PE, contiguous DMA out.
"""

import sys

if "/opt/trn_rl_repo" not in sys.path:
    sys.path.insert(0, "/opt/trn_rl_repo")

import numpy as np
import ml_dtypes

import concourse.bass as bass
import concourse.tile as tile
from concourse import bacc, mybir
from concourse.bass import ds, ts
from concourse.masks import make_identity

# ---------------------------------------------------------------- constants
B, S, D, VOCAB, HID = 32, 2048, 64, 10000, 256
CORES = 8
NSEQ = B // CORES          # 4 sequences per core
P = 128
NT = S // P                # 16 t-blocks of 128
SCALE = 1.0 / float(np.sqrt(np.float32(64.0)))

F32 = mybir.dt.float32
BF16 = mybir.dt.bfloat16
FP8 = mybir.dt.float8e4
I16 = mybir.dt.int16
VAU_W = 80  # Vau row stride: DoubleRow needs the interleave step %16 == 0




_LDW_PATCHED = False


def _enable_walrus_ldw_opt():
    """Flip walrus's disabled-by-default LDWEIGHTS optimization pass."""
    global _LDW_PATCHED
    if _LDW_PATCHED:
        return
    from concourse import bass_utils as bu

    orig = bu.run_command

    def patched(cmd, **kw):
        cmd = ["--enable-ldw-opt=true" if c == "--enable-ldw-opt=false" else c
               for c in cmd]
        return orig(cmd, **kw)

    bu.run_command = patched
    _LDW_PATCHED = True


def build(n_seq: int = NSEQ):
    """Build + compile the per-core Bass program (identical on all cores)."""
    import os

    if os.environ.get("BASS_LDW_OPT"):
        _enable_walrus_ldw_opt()
    nc = bacc.Bacc("TRN2", target_bir_lowering=False, debug=False)

    xg_d = nc.dram_tensor("xg", [n_seq, P, S // 16], I16, kind="ExternalInput")
    embp_d = nc.dram_tensor("embp", [VOCAB, P], BF16, kind="ExternalInput")
    wqa_d = nc.dram_tensor("wqa", [D + 1, D], BF16, kind="ExternalInput")
    wka_d = nc.dram_tensor("wka", [D + 1, D], BF16, kind="ExternalInput")
    wv_d = nc.dram_tensor("wv", [D, D], BF16, kind="ExternalInput")
    w1a_d = nc.dram_tensor("w1a", [D + 1, HID], BF16, kind="ExternalInput")
    w2_d = nc.dram_tensor("w2", [HID, D], BF16, kind="ExternalInput")
    b2_d = nc.dram_tensor("b2", [D, 1], F32, kind="ExternalInput")
    out_d = nc.dram_tensor("out", [n_seq, S, D], F32, kind="ExternalOutput")

    with tile.TileContext(nc) as tc:
        _emit(nc, tc, n_seq, xg_d, embp_d, wqa_d, wka_d, wv_d, w1a_d, w2_d,
              b2_d, out_d)

    nc.compile()
    return nc


def _emit(nc, tc, n_seq, xg_d, embp_d, wqa_d, wka_d, wv_d, w1a_d, w2_d,
          b2_d, out_d):
    from contextlib import ExitStack

    ctx = ExitStack()
    const = ctx.enter_context(tc.tile_pool(name="const", bufs=1))
    sb = ctx.enter_context(tc.tile_pool(name="sb", bufs=2))
    # PSUM pools: scores 2x2 banks + upd 2 + misc 2 = 8 banks exactly.
    scp = ctx.enter_context(tc.tile_pool(name="scp", bufs=2, space="PSUM"))
    updp = ctx.enter_context(tc.tile_pool(name="updp", bufs=2, space="PSUM"))
    miscp = ctx.enter_context(tc.tile_pool(name="miscp", bufs=2, space="PSUM"))

    # ---- constants ----
    wqa_t = const.tile([D + 1, D], BF16)
    wka_t = const.tile([D + 1, D], BF16)
    wv_t = const.tile([D, D], BF16)
    w1a_t = const.tile([D + 1, HID], BF16)
    w2_t = const.tile([P, 2, D], BF16)
    b2_t = const.tile([D, 1], F32)
    ident_t = const.tile([D, D], F32)
    ones_t = const.tile([1, D + 1], BF16)
    nc.vector.memset(ones_t, 1.0)
    nc.sync.dma_start(out=wqa_t, in_=wqa_d.ap())
    nc.sync.dma_start(out=wka_t, in_=wka_d.ap())
    nc.sync.dma_start(out=wv_t, in_=wv_d.ap())
    nc.sync.dma_start(out=w1a_t, in_=w1a_d.ap())
    nc.sync.dma_start(out=w2_t, in_=w2_d.ap().rearrange("(h p) d -> p h d", p=P))
    nc.sync.dma_start(out=b2_t, in_=b2_d.ap())
    make_identity(nc, ident_t)

    # HAM warmup: ~5us of dense dummy matmuls so the PE clock-gate opens
    # (K=8/8) before the real work; scattered sub-us gaps then keep it open.
    wup = miscp.tile([P, 512], F32, name="wup", tag="misc")
    for _ in range(24):
        nc.tensor.matmul(out=wup[0:D, 0:256], lhsT=wqa_t, rhs=w1a_t,
                         start=True, stop=True)
    # preload the exp activation table (~2.7us) off the critical path
    wux = const.tile([1, D], F32)
    nc.scalar.activation(out=wux, in_=wup[0:1, 0:D],
                         func=mybir.ActivationFunctionType.Exp, scale=0.001)

    # ---- per-seq state (python handles) ----
    idx = [None] * n_seq
    eTa = [None] * n_seq     # [128, S] bf16  (rows 0-63 e^T, row 64 ones)
    QT2 = [None] * n_seq     # [128, S] bf16  (rows 64-127 duplicate 0-63)
    KT2 = [None] * n_seq
    Vau = [None] * n_seq     # [128, NT, 65] bf16 (col 0 ones)
    expT = [None] * n_seq    # [128, NT, S] bf16
    updps = [dict() for _ in range(n_seq)]   # (j) -> psum tile [65, 512]
    updn = [dict() for _ in range(n_seq)]    # (j) -> sbuf [65, 512] bf16

    def proj_chunk(k, c):
        """Gather + Q^T/K^T (duplicated) + V_aug for 512-token chunk c of
        sequence k.  Chunk c provides QT2/KT2[:, 512c:512c+512] and V_aug
        t-blocks 4c..4c+3, so attention rounds can chase chunks."""
        if c == 0:
            idx[k] = sb.tile([P, S // 16], I16, name=f"idx{k}", tag="idx",
                             bufs=2)
            nc.sync.dma_start(out=idx[k], in_=xg_d.ap()[k])
            eTa[k] = sb.tile([P, S], BF16, name=f"eTa{k}", tag="eTa", bufs=2)
            QT2[k] = sb.tile([P, S], BF16, name=f"QT2{k}", tag="QT2", bufs=2)
            KT2[k] = sb.tile([P, S], BF16, name=f"KT2{k}", tag="KT2", bufs=2)
            # col 0 = ones: the softmax denominator rides as upd row 0.
            Vau[k] = sb.tile([P, NT, D + 1], BF16, name=f"Vau{k}", tag="Vau",
                             bufs=2)
            nc.vector.memset(Vau[k][:, :, 0:1], 1.0)
            expT[k] = sb.tile([P, NT, S], BF16, name=f"expT{k}", tag="expT",
                              bufs=2)
        CH = 512  # >512 idxs per gather overflows the SWDGE queue
        nc.gpsimd.dma_gather(
            out_ap=eTa[k][:, c * CH : (c + 1) * CH].unsqueeze(1),
            in_ap=embp_d.ap(),
            idxs_ap=idx[k][:, c * (CH // 16) : (c + 1) * (CH // 16)],
            num_idxs=CH,
            num_idxs_reg=CH,
            elem_size=P,
            transpose=True,
        )
        for w_t, dst in ((wqa_t, QT2[k]), (wka_t, KT2[k])):
            pps = miscp.tile([P, 512], F32, name=f"pps{k}_{c}", tag="misc")
            rhs = eTa[k][0 : D + 1, ts(c, 512)]
            nc.tensor.matmul(out=pps[0:D, :], lhsT=w_t, rhs=rhs,
                             start=True, stop=True, tile_position=(0, 0))
            nc.tensor.matmul(out=pps[D:P, :], lhsT=w_t, rhs=rhs,
                             start=True, stop=True, tile_position=(0, 64))
            nc.vector.tensor_copy(out=dst[:, ts(c, 512)], in_=pps)
        for i in range(4 * c, 4 * c + 4):
            vps = miscp.tile([P, D], F32, name=f"vps{k}_{i}", tag="misc")
            nc.tensor.matmul(out=vps, lhsT=eTa[k][0:D, ts(i, P)], rhs=wv_t,
                             start=True, stop=True)
            nc.vector.tensor_copy(out=Vau[k][:, i, 1 : D + 1], in_=vps)

    def att_round(k, p, sq):
        """Scores for t-blocks (2p, 2p+1) x s-quarter sq + exp."""
        t0, t1 = 2 * p, 2 * p + 1
        sc = scp.tile([P, 1024], F32, name=f"sc{k}_{sq}_{p}", tag="sc")
        nc.tensor.matmul(out=sc[:, 0:512],
                         lhsT=KT2[k][0:D, ts(t0, P)],
                         rhs=QT2[k][0:D, ts(sq, 512)],
                         start=True, stop=True, tile_position=(0, 0))
        nc.tensor.matmul(out=sc[:, 512:1024],
                         lhsT=KT2[k][D:P, ts(t1, P)],
                         rhs=QT2[k][D:P, ts(sq, 512)],
                         start=True, stop=True, tile_position=(64, 0))
        nc.scalar.activation(
            out=expT[k][:, t0 : t0 + 2, ts(sq, 512)],
            in_=sc.rearrange("x (a b) -> x a b", a=2),
            func=mybir.ActivationFunctionType.Exp,
            scale=SCALE,
        )

    def upd_round(k, p, sq):
        """upd accumulation MMs for a round whose exp finished ~2 rounds ago
        (the lag keeps these off the scores->exp dependency chain)."""
        j = sq
        if j not in updps[k]:
            updps[k][j] = updp.tile([D + 1, 512], F32,
                                    name=f"upd{k}_{j}", tag="upd")
        for t in (2 * p, 2 * p + 1):
            nc.tensor.matmul(out=updps[k][j],
                             lhsT=Vau[k][:, t, 0 : D + 1],
                             rhs=expT[k][:, t, ts(j, 512)],
                             start=(t == 0), stop=(t == NT - 1))
        if p == 7:
            normalize(k, sq)

    def normalize(k, j):
        """updn = upd / denom (denom = row 0); row 0 becomes ~1.0.

        The reciprocal is computed on the [1, 512] denominator row, then
        replicated across partitions with a K=1 fp32 ones-matmul on the PE
        (DVE lanes cannot cross partitions and gpsimd broadcast is slow)."""
        ups = updps[k].pop(j)
        updu = sb.tile([D + 1, 512], F32, name=f"updu{k}_{j}", tag="updu",
                       bufs=2)
        nc.vector.tensor_copy(out=updu, in_=ups)
        recd = sb.tile([1, 512], F32, name=f"recd{k}_{j}", tag="recd", bufs=2)
        nc.vector.reciprocal_approx_fast(out=recd, in_=updu[0:1, :])
        recb = sb.tile([1, 512], BF16, name=f"recb{k}_{j}", tag="recb", bufs=2)
        nc.vector.tensor_copy(out=recb, in_=recd)
        rec_ps = miscp.tile([D + 1, 512], F32, name=f"recps{k}_{j}",
                            tag="misc")
        nc.tensor.matmul(out=rec_ps, lhsT=ones_t, rhs=recb,
                         start=True, stop=True)
        updn[k][j] = sb.tile([D + 1, 512], BF16, name=f"updn{k}_{j}",
                             tag="updn", bufs=8)
        nc.vector.tensor_mul(out=updn[k][j], in0=updu, in1=rec_ps)

    def ffn(k, j):
        """relu FFN + out-proj + transpose + store for s-block j of seq k."""
        un = updn[k].pop(j)
        hn = []
        for half in range(2):
            hps = miscp.tile([P, 512], F32, name=f"hps{k}_{j}_{half}",
                             tag="misc")
            nc.tensor.matmul(out=hps, lhsT=w1a_t[:, ts(half, P)], rhs=un,
                             start=True, stop=True)
            hnt = sb.tile([P, 512], BF16, name=f"hn{k}_{j}_{half}", tag="hn",
                          bufs=4)
            nc.vector.tensor_scalar(out=hnt, in0=hps, scalar1=0.0,
                                    scalar2=None, op0=mybir.AluOpType.max)
            hn.append(hnt)
        ops = miscp.tile([D, 512], F32, name=f"ops{k}_{j}", tag="misc")
        for half in range(2):
            nc.tensor.matmul(out=ops, lhsT=w2_t[:, half, :], rhs=hn[half],
                             start=(half == 0), stop=(half == 1))
        outT = sb.tile([D, 512], F32, name=f"outT{k}_{j}", tag="outT", bufs=2)
        nc.vector.tensor_scalar(out=outT, in0=ops, scalar1=b2_t,
                                scalar2=None, op0=mybir.AluOpType.add)
        ob = sb.tile([P, 4, D], F32, name=f"ob{k}_{j}", tag="ob", bufs=2)
        for q in range(4):
            tps = miscp.tile([P, D], F32, name=f"tps{k}_{j}_{q}", tag="misc")
            nc.tensor.transpose(out=tps, in_=outT[:, ts(q, P)],
                                identity=ident_t)
            nc.vector.tensor_copy(out=ob[:, q, :], in_=tps)
        dst = out_d.ap()[k][ts(j, 512)].rearrange("(q p) d -> p q d", p=P)
        nc.sync.dma_start(out=dst, in_=ob)

    # ------------------------------- schedule -------------------------------
    # s-quarter outer, t-pair inner; one upd accumulation group at a time.
    # seq 0's projection chunks are emitted just ahead of the rounds that
    # first need them (round (sq=0, p) needs KT2/V chunk p//2); seq k+1's
    # chunks are prefetched inside att(k)'s sq=1 rounds.  FFN for quarter j
    # runs interleaved into quarter j+1's rounds; ffn(k, 3) lands in
    # att(k+1)'s rounds.
    for k in range(n_seq):
        for sq in range(4):
            for p in range(8):
                if k == 0 and sq == 0 and p % 2 == 0:
                    proj_chunk(0, p // 2)
                att_round(k, p, sq)
                if p == 2:
                    if sq == 0 and k > 0:
                        ffn(k - 1, 3)
                    elif sq >= 1:
                        ffn(k, sq - 1)
                if sq == 1 and p % 2 == 1 and k + 1 < n_seq:
                    proj_chunk(k + 1, p // 2)
                if p == 7:
                    normalize(k, sq)
    ffn(n_seq - 1, 3)
    ctx.close()


# ---------------------------------------------------------------- host side

def _prep_params(inputs):
    """Host-side parameter prep (layout changes + bias folds only)."""
    f = {k: np.asarray(v) for k, v in inputs.items()}
    emb = f["emb"].astype(np.float32)
    embp = np.zeros((VOCAB, P), dtype=ml_dtypes.bfloat16)
    embp[:, :D] = emb.astype(ml_dtypes.bfloat16)
    embp[:, D] = 1.0
    wqa = np.concatenate([f["wq"], f["bq"][None, :]], 0).astype(ml_dtypes.bfloat16)
    wka = np.concatenate([f["wk"], f["bk"][None, :]], 0).astype(ml_dtypes.bfloat16)
    b1p = (f["b1"].astype(np.float64)
           + f["bv"].astype(np.float64) @ f["w1"].astype(np.float64))
    # bias row FIRST: updn row 0 is the (denom * 1/denom) ~= 1.0 ones row
    w1a = np.concatenate([b1p[None, :].astype(np.float32), f["w1"]], 0).astype(
        ml_dtypes.bfloat16)
    return {
        "embp": embp,
        "wqa": wqa,
        "wka": wka,
        "wv": f["wv"].astype(ml_dtypes.bfloat16),
        "w1a": w1a,
        "w2": f["w2"].astype(ml_dtypes.bfloat16),
        "b2": f["b2"].astype(np.float32).reshape(D, 1),
    }


def _prep_xg(x_shard):
    """Pack token ids into dma_gather's index layout: [seq, 128, S//16] i16,
    idx[p, f] = x[f*16 + p%16], replicated across the 8 Q7 core stripes."""
    n_seq = x_shard.shape[0]
    xg = np.empty((n_seq, P, S // 16), dtype=np.int16)
    for s_i in range(n_seq):
        m = x_shard[s_i].astype(np.int16).reshape(S // 16, 16).T  # [16, S/16]
        xg[s_i] = np.tile(m, (8, 1))
    return xg


_CACHED_NC = None
LAST_EXEC_NS = None


def _install_ntff_hook():
    """Expose the axon NTFF profiling hook that bass_utils looks for."""
    import types

    if "antenv.axon_hooks" in sys.modules:
        return
    try:
        from trn_agent_boot.trn_boot import _ntff_profile_via_ctypes

        hook = _ntff_profile_via_ctypes("/opt/axon/libaxon_pjrt.so")
    except Exception:
        return
    m = types.ModuleType("antenv.axon_hooks")
    m.get_axon_ntff_profile_hook = lambda: hook
    m.set_axon_ntff_profile_hook = lambda h: None
    sys.modules["antenv.axon_hooks"] = m


def kernel(**inputs) -> np.ndarray:
    global _CACHED_NC, LAST_EXEC_NS
    import os
    from concourse import bass_utils

    params = _prep_params(inputs)
    x = np.asarray(inputs["x"]).astype(np.int64)
    assert x.shape == (B, S)

    if _CACHED_NC is None:
        _CACHED_NC = build(NSEQ)
    nc = _CACHED_NC

    in_maps = []
    for c in range(CORES):
        shard = x[c * NSEQ : (c + 1) * NSEQ]
        m = dict(params)
        m["xg"] = _prep_xg(shard)
        in_maps.append(m)

    trace = bool(os.environ.get("BASS_KERNEL_TRACE"))
    kw = {}
    if trace:
        _install_ntff_hook()
        kw = {"trace": True,
              "tmpdir": os.environ.get("BASS_KERNEL_TRACE_DIR") or None}
    res = bass_utils.run_bass_kernel_spmd(nc, in_maps,
                                          core_ids=list(range(CORES)), **kw)
    LAST_EXEC_NS = res.exec_time_ns
    out = np.concatenate([r["out"] for r in res.results], axis=0)
    return out.astype(np.float32)


# revision 31
# speedup vs baseline: 1.0958x; 1.0958x over previous
"""Trainium2 Bass kernel for nn_Encoder_9818295238798.

Single-layer transformer encoder: embed -> QKV -> full softmax attention
-> 2-layer FFN (relu).  B=32, S=2048, D=64, VOCAB=10000.

Sharding: data-parallel over the batch dim, 4 sequences per core x 8 cores,
parameters replicated.

Layout strategy: everything on-chip lives transposed ([feature, token]) so
that every bias is a per-partition scalar and softmax reductions land in the
matmul contraction:
  - embedding rows are gathered pre-transposed via dma_gather(transpose=True)
    from a host-prepared bf16 emb padded to 128 cols (col 64 = 1.0 so the
    "ones row" used for the Q/K bias-fold comes along for free)
  - Q^T/K^T computed with the weight (with bias row appended) stationary,
    duplicated into partitions 0-63 / 64-127 via PE col-tiling so the scores
    matmuls can run 2-way row-packed (K=64 only half-fills the PE array)
  - scores computed transposed [t, s]; exp on ScalarE straight out of PSUM
    with the 1/sqrt(D) scale folded into the activation's free scale field
    (scores are bounded ~+-3 so no max-subtraction is needed)
  - softmax denominator rides along as a 65th (ones) column of V
  - upd normalization via a partition-broadcast of the reciprocal; V bias is
    folded into b1 on the host (rows of probs sum to 1)
  - FFN biases ride the contraction as appended ones rows; final transpose
    back on the PE, contiguous DMA out.
"""

import sys

if "/opt/trn_rl_repo" not in sys.path:
    sys.path.insert(0, "/opt/trn_rl_repo")

import numpy as np
import ml_dtypes

import concourse.bass as bass
import concourse.tile as tile
from concourse import bacc, mybir
from concourse.bass import ds, ts
from concourse.masks import make_identity

# ---------------------------------------------------------------- constants
B, S, D, VOCAB, HID = 32, 2048, 64, 10000, 256
CORES = 8
NSEQ = B // CORES          # 4 sequences per core
P = 128
NT = S // P                # 16 t-blocks of 128
SCALE = 1.0 / float(np.sqrt(np.float32(64.0)))

F32 = mybir.dt.float32
BF16 = mybir.dt.bfloat16
FP8 = mybir.dt.float8e4
I16 = mybir.dt.int16
VAU_W = 80  # Vau row stride: DoubleRow needs the interleave step %16 == 0




_LDW_PATCHED = False


def _enable_walrus_ldw_opt():
    """Flip walrus's disabled-by-default LDWEIGHTS optimization pass."""
    global _LDW_PATCHED
    if _LDW_PATCHED:
        return
    from concourse import bass_utils as bu

    orig = bu.run_command

    def patched(cmd, **kw):
        cmd = ["--enable-ldw-opt=true" if c == "--enable-ldw-opt=false" else c
               for c in cmd]
        return orig(cmd, **kw)

    bu.run_command = patched
    _LDW_PATCHED = True


def build(n_seq: int = NSEQ):
    """Build + compile the per-core Bass program (identical on all cores)."""
    import os

    if os.environ.get("BASS_LDW_OPT"):
        _enable_walrus_ldw_opt()
    nc = bacc.Bacc("TRN2", target_bir_lowering=False, debug=False)

    xg_d = nc.dram_tensor("xg", [n_seq, P, S // 16], I16, kind="ExternalInput")
    embp_d = nc.dram_tensor("embp", [VOCAB, P], BF16, kind="ExternalInput")
    wqa_d = nc.dram_tensor("wqa", [D + 1, D], BF16, kind="ExternalInput")
    wka_d = nc.dram_tensor("wka", [D + 1, D], BF16, kind="ExternalInput")
    wv_d = nc.dram_tensor("wv", [D, D], BF16, kind="ExternalInput")
    w1a_d = nc.dram_tensor("w1a", [D + 1, HID], BF16, kind="ExternalInput")
    w2_d = nc.dram_tensor("w2", [HID, D], BF16, kind="ExternalInput")
    b2_d = nc.dram_tensor("b2", [D, 1], F32, kind="ExternalInput")
    out_d = nc.dram_tensor("out", [n_seq, S, D], F32, kind="ExternalOutput")

    with tile.TileContext(nc) as tc:
        _emit(nc, tc, n_seq, xg_d, embp_d, wqa_d, wka_d, wv_d, w1a_d, w2_d,
              b2_d, out_d)

    nc.compile()
    return nc


def _emit(nc, tc, n_seq, xg_d, embp_d, wqa_d, wka_d, wv_d, w1a_d, w2_d,
          b2_d, out_d):
    from contextlib import ExitStack

    ctx = ExitStack()
    const = ctx.enter_context(tc.tile_pool(name="const", bufs=1))
    sb = ctx.enter_context(tc.tile_pool(name="sb", bufs=2))
    # PSUM pools: scores 2x2 banks + upd 2 + misc 2 = 8 banks exactly.
    scp = ctx.enter_context(tc.tile_pool(name="scp", bufs=2, space="PSUM"))
    updp = ctx.enter_context(tc.tile_pool(name="updp", bufs=2, space="PSUM"))
    miscp = ctx.enter_context(tc.tile_pool(name="miscp", bufs=2, space="PSUM"))

    # ---- constants ----
    wqa_t = const.tile([D + 1, D], BF16)
    wka_t = const.tile([D + 1, D], BF16)
    wv_t = const.tile([D, D], BF16)
    w1a_t = const.tile([D + 1, HID], BF16)
    w2_t = const.tile([P, 2, D], BF16)
    b2_t = const.tile([D, 1], F32)
    ident_t = const.tile([D, D], F32)
    ones_t = const.tile([1, D + 1], BF16)
    nc.vector.memset(ones_t, 1.0)
    nc.sync.dma_start(out=wqa_t, in_=wqa_d.ap())
    nc.sync.dma_start(out=wka_t, in_=wka_d.ap())
    nc.sync.dma_start(out=wv_t, in_=wv_d.ap())
    nc.sync.dma_start(out=w1a_t, in_=w1a_d.ap())
    nc.sync.dma_start(out=w2_t, in_=w2_d.ap().rearrange("(h p) d -> p h d", p=P))
    nc.sync.dma_start(out=b2_t, in_=b2_d.ap())
    make_identity(nc, ident_t)

    # HAM warmup: ~5us of dense dummy matmuls so the PE clock-gate opens
    # (K=8/8) before the real work; scattered sub-us gaps then keep it open.
    wup = miscp.tile([P, 512], F32, name="wup", tag="misc")
    for _ in range(24):
        nc.tensor.matmul(out=wup[0:D, 0:256], lhsT=wqa_t, rhs=w1a_t,
                         start=True, stop=True)
    # preload the exp activation table (~2.7us) off the critical path
    wux = const.tile([1, D], F32)
    nc.scalar.activation(out=wux, in_=wup[0:1, 0:D],
                         func=mybir.ActivationFunctionType.Exp, scale=0.001)

    # ---- per-seq state (python handles) ----
    idx = [None] * n_seq
    eTa = [None] * n_seq     # [128, S] bf16  (rows 0-63 e^T, row 64 ones)
    QT2 = [None] * n_seq     # [128, S] bf16  (rows 64-127 duplicate 0-63)
    KT2 = [None] * n_seq
    Vau = [None] * n_seq     # [128, NT, 65] bf16 (col 0 ones)
    expT = [None] * n_seq    # [128, NT, S] bf16
    updps = [dict() for _ in range(n_seq)]   # (j) -> psum tile [65, 512]
    updn = [dict() for _ in range(n_seq)]    # (j) -> sbuf [65, 512] bf16

    def proj_chunk(k, c):
        """Gather + Q^T/K^T (duplicated) + V_aug for 512-token chunk c of
        sequence k.  Chunk c provides QT2/KT2[:, 512c:512c+512] and V_aug
        t-blocks 4c..4c+3, so attention rounds can chase chunks."""
        if c == 0:
            idx[k] = sb.tile([P, S // 16], I16, name=f"idx{k}", tag="idx",
                             bufs=2)
            nc.sync.dma_start(out=idx[k], in_=xg_d.ap()[k])
            eTa[k] = sb.tile([P, S], BF16, name=f"eTa{k}", tag="eTa", bufs=2)
            QT2[k] = sb.tile([P, S], BF16, name=f"QT2{k}", tag="QT2", bufs=2)
            KT2[k] = sb.tile([P, S], BF16, name=f"KT2{k}", tag="KT2", bufs=2)
            # col 0 = ones: the softmax denominator rides as upd row 0.
            Vau[k] = sb.tile([P, NT, D + 1], BF16, name=f"Vau{k}", tag="Vau",
                             bufs=2)
            nc.vector.memset(Vau[k][:, :, 0:1], 1.0)
            expT[k] = sb.tile([P, NT, S], BF16, name=f"expT{k}", tag="expT",
                              bufs=2)
        CH = 512  # >512 idxs per gather overflows the SWDGE queue
        nc.gpsimd.dma_gather(
            out_ap=eTa[k][:, c * CH : (c + 1) * CH].unsqueeze(1),
            in_ap=embp_d.ap(),
            idxs_ap=idx[k][:, c * (CH // 16) : (c + 1) * (CH // 16)],
            num_idxs=CH,
            num_idxs_reg=CH,
            elem_size=P,
            transpose=True,
        )
        for w_t, dst in ((wqa_t, QT2[k]), (wka_t, KT2[k])):
            pps = miscp.tile([P, 512], F32, name=f"pps{k}_{c}", tag="misc")
            rhs = eTa[k][0 : D + 1, ts(c, 512)]
            nc.tensor.matmul(out=pps[0:D, :], lhsT=w_t, rhs=rhs,
                             start=True, stop=True, tile_position=(0, 0))
            nc.tensor.matmul(out=pps[D:P, :], lhsT=w_t, rhs=rhs,
                             start=True, stop=True, tile_position=(0, 64))
            nc.vector.tensor_copy(out=dst[:, ts(c, 512)], in_=pps)
        for i in range(4 * c, 4 * c + 4):
            vps = miscp.tile([P, D], F32, name=f"vps{k}_{i}", tag="misc")
            nc.tensor.matmul(out=vps, lhsT=eTa[k][0:D, ts(i, P)], rhs=wv_t,
                             start=True, stop=True)
            nc.vector.tensor_copy(out=Vau[k][:, i, 1 : D + 1], in_=vps)

    def att_round(k, p, sq):
        """Scores for t-blocks (2p, 2p+1) x s-quarter sq + exp."""
        t0, t1 = 2 * p, 2 * p + 1
        sc = scp.tile([P, 1024], F32, name=f"sc{k}_{sq}_{p}", tag="sc")
        nc.tensor.matmul(out=sc[:, 0:512],
                         lhsT=KT2[k][0:D, ts(t0, P)],
                         rhs=QT2[k][0:D, ts(sq, 512)],
                         start=True, stop=True, tile_position=(0, 0))
        nc.tensor.matmul(out=sc[:, 512:1024],
                         lhsT=KT2[k][D:P, ts(t1, P)],
                         rhs=QT2[k][D:P, ts(sq, 512)],
                         start=True, stop=True, tile_position=(64, 0))
        nc.scalar.activation(
            out=expT[k][:, t0 : t0 + 2, ts(sq, 512)],
            in_=sc.rearrange("x (a b) -> x a b", a=2),
            func=mybir.ActivationFunctionType.Exp,
            scale=SCALE,
        )

    def upd_round(k, p, sq):
        """upd accumulation MMs for a round whose exp finished ~2 rounds ago
        (the lag keeps these off the scores->exp dependency chain)."""
        j = sq
        if j not in updps[k]:
            updps[k][j] = updp.tile([D + 1, 512], F32,
                                    name=f"upd{k}_{j}", tag="upd")
        for t in (2 * p, 2 * p + 1):
            nc.tensor.matmul(out=updps[k][j],
                             lhsT=Vau[k][:, t, 0 : D + 1],
                             rhs=expT[k][:, t, ts(j, 512)],
                             start=(t == 0), stop=(t == NT - 1))
        if p == 7:
            normalize(k, sq)

    def normalize(k, j):
        """updn = upd / denom (denom = row 0); row 0 becomes ~1.0.

        The reciprocal is computed on the [1, 512] denominator row, then
        replicated across partitions with a K=1 fp32 ones-matmul on the PE
        (DVE lanes cannot cross partitions and gpsimd broadcast is slow)."""
        ups = updps[k].pop(j)
        updu = sb.tile([D + 1, 512], F32, name=f"updu{k}_{j}", tag="updu",
                       bufs=2)
        nc.vector.tensor_copy(out=updu, in_=ups)
        recd = sb.tile([1, 512], F32, name=f"recd{k}_{j}", tag="recd", bufs=2)
        nc.vector.reciprocal_approx_fast(out=recd, in_=updu[0:1, :])
        recb = sb.tile([1, 512], BF16, name=f"recb{k}_{j}", tag="recb", bufs=2)
        nc.vector.tensor_copy(out=recb, in_=recd)
        rec_ps = miscp.tile([D + 1, 512], F32, name=f"recps{k}_{j}",
                            tag="misc")
        nc.tensor.matmul(out=rec_ps, lhsT=ones_t, rhs=recb,
                         start=True, stop=True)
        updn[k][j] = sb.tile([D + 1, 512], BF16, name=f"updn{k}_{j}",
                             tag="updn", bufs=8)
        nc.vector.tensor_mul(out=updn[k][j], in0=updu, in1=rec_ps)

    def ffn(k, j):
        """relu FFN + out-proj + transpose + store for s-block j of seq k."""
        un = updn[k].pop(j)
        hn = []
        for half in range(2):
            hps = miscp.tile([P, 512], F32, name=f"hps{k}_{j}_{half}",
                             tag="misc")
            nc.tensor.matmul(out=hps, lhsT=w1a_t[:, ts(half, P)], rhs=un,
                             start=True, stop=True)
            hnt = sb.tile([P, 512], BF16, name=f"hn{k}_{j}_{half}", tag="hn",
                          bufs=4)
            nc.vector.tensor_scalar(out=hnt, in0=hps, scalar1=0.0,
                                    scalar2=None, op0=mybir.AluOpType.max)
            hn.append(hnt)
        ops = miscp.tile([D, 512], F32, name=f"ops{k}_{j}", tag="misc")
        for half in range(2):
            nc.tensor.matmul(out=ops, lhsT=w2_t[:, half, :], rhs=hn[half],
                             start=(half == 0), stop=(half == 1))
        outT = sb.tile([D, 512], F32, name=f"outT{k}_{j}", tag="outT", bufs=2)
        nc.vector.tensor_scalar(out=outT, in0=ops, scalar1=b2_t,
                                scalar2=None, op0=mybir.AluOpType.add)
        ob = sb.tile([P, 4, D], F32, name=f"ob{k}_{j}", tag="ob", bufs=2)
        for q in range(4):
            tps = miscp.tile([P, D], F32, name=f"tps{k}_{j}_{q}", tag="misc")
            nc.tensor.transpose(out=tps, in_=outT[:, ts(q, P)],
                                identity=ident_t)
            nc.vector.tensor_copy(out=ob[:, q, :], in_=tps)
        dst = out_d.ap()[k][ts(j, 512)].rearrange("(q p) d -> p q d", p=P)
        nc.sync.dma_start(out=dst, in_=ob)

    # ------------------------------- schedule -------------------------------
    # s-quarter outer, t-pair inner.  upd MMs trail their round by UPD_LAG
    # rounds so the scores->exp chain never waits on them; normalize rides
    # the lagged upd close.  seq 0's projection chunks are emitted just ahead
    # of the rounds that first need them; seq k+1's chunks are prefetched
    # inside att(k)'s sq=1 rounds.  FFN for quarter j runs interleaved into
    # quarter j+1's rounds; ffn(k, 3) lands in att(k+1)'s rounds.
    from collections import deque

    UPD_LAG = 2
    pending = deque()
    for k in range(n_seq):
        for sq in range(4):
            for p in range(8):
                if k == 0 and sq == 0 and p % 2 == 0:
                    proj_chunk(0, p // 2)
                att_round(k, p, sq)
                pending.append((k, p, sq))
                if len(pending) > UPD_LAG:
                    upd_round(*pending.popleft())
                if p == 2:
                    if sq == 0 and k > 0:
                        ffn(k - 1, 3)
                    elif sq >= 1:
                        ffn(k, sq - 1)
                if sq == 1 and p % 2 == 1 and k + 1 < n_seq:
                    proj_chunk(k + 1, p // 2)
    while pending:
        upd_round(*pending.popleft())
    ffn(n_seq - 1, 3)
    ctx.close()


# ---------------------------------------------------------------- host side

def _prep_params(inputs):
    """Host-side parameter prep (layout changes + bias folds only)."""
    f = {k: np.asarray(v) for k, v in inputs.items()}
    emb = f["emb"].astype(np.float32)
    embp = np.zeros((VOCAB, P), dtype=ml_dtypes.bfloat16)
    embp[:, :D] = emb.astype(ml_dtypes.bfloat16)
    embp[:, D] = 1.0
    wqa = np.concatenate([f["wq"], f["bq"][None, :]], 0).astype(ml_dtypes.bfloat16)
    wka = np.concatenate([f["wk"], f["bk"][None, :]], 0).astype(ml_dtypes.bfloat16)
    b1p = (f["b1"].astype(np.float64)
           + f["bv"].astype(np.float64) @ f["w1"].astype(np.float64))
    # bias row FIRST: updn row 0 is the (denom * 1/denom) ~= 1.0 ones row
    w1a = np.concatenate([b1p[None, :].astype(np.float32), f["w1"]], 0).astype(
        ml_dtypes.bfloat16)
    return {
        "embp": embp,
        "wqa": wqa,
        "wka": wka,
        "wv": f["wv"].astype(ml_dtypes.bfloat16),
        "w1a": w1a,
        "w2": f["w2"].astype(ml_dtypes.bfloat16),
        "b2": f["b2"].astype(np.float32).reshape(D, 1),
    }


def _prep_xg(x_shard):
    """Pack token ids into dma_gather's index layout: [seq, 128, S//16] i16,
    idx[p, f] = x[f*16 + p%16], replicated across the 8 Q7 core stripes."""
    n_seq = x_shard.shape[0]
    xg = np.empty((n_seq, P, S // 16), dtype=np.int16)
    for s_i in range(n_seq):
        m = x_shard[s_i].astype(np.int16).reshape(S // 16, 16).T  # [16, S/16]
        xg[s_i] = np.tile(m, (8, 1))
    return xg


_CACHED_NC = None
LAST_EXEC_NS = None


def _install_ntff_hook():
    """Expose the axon NTFF profiling hook that bass_utils looks for."""
    import types

    if "antenv.axon_hooks" in sys.modules:
        return
    try:
        from trn_agent_boot.trn_boot import _ntff_profile_via_ctypes

        hook = _ntff_profile_via_ctypes("/opt/axon/libaxon_pjrt.so")
    except Exception:
        return
    m = types.ModuleType("antenv.axon_hooks")
    m.get_axon_ntff_profile_hook = lambda: hook
    m.set_axon_ntff_profile_hook = lambda h: None
    sys.modules["antenv.axon_hooks"] = m


def kernel(**inputs) -> np.ndarray:
    global _CACHED_NC, LAST_EXEC_NS
    import os
    from concourse import bass_utils

    params = _prep_params(inputs)
    x = np.asarray(inputs["x"]).astype(np.int64)
    assert x.shape == (B, S)

    if _CACHED_NC is None:
        _CACHED_NC = build(NSEQ)
    nc = _CACHED_NC

    in_maps = []
    for c in range(CORES):
        shard = x[c * NSEQ : (c + 1) * NSEQ]
        m = dict(params)
        m["xg"] = _prep_xg(shard)
        in_maps.append(m)

    trace = bool(os.environ.get("BASS_KERNEL_TRACE"))
    kw = {}
    if trace:
        _install_ntff_hook()
        kw = {"trace": True,
              "tmpdir": os.environ.get("BASS_KERNEL_TRACE_DIR") or None}
    res = bass_utils.run_bass_kernel_spmd(nc, in_maps,
                                          core_ids=list(range(CORES)), **kw)
    LAST_EXEC_NS = res.exec_time_ns
    out = np.concatenate([r["out"] for r in res.results], axis=0)
    return out.astype(np.float32)


# revision 33
# speedup vs baseline: 1.0988x; 1.0027x over previous
"""Trainium2 Bass kernel for nn_Encoder_9818295238798.

Single-layer transformer encoder: embed -> QKV -> full softmax attention
-> 2-layer FFN (relu).  B=32, S=2048, D=64, VOCAB=10000.

Sharding: data-parallel over the batch dim, 4 sequences per core x 8 cores,
parameters replicated.

Layout strategy: everything on-chip lives transposed ([feature, token]) so
that every bias is a per-partition scalar and softmax reductions land in the
matmul contraction:
  - embedding rows are gathered pre-transposed via dma_gather(transpose=True)
    from a host-prepared bf16 emb padded to 128 cols (col 64 = 1.0 so the
    "ones row" used for the Q/K bias-fold comes along for free)
  - Q^T/K^T computed with the weight (with bias row appended) stationary,
    duplicated into partitions 0-63 / 64-127 via PE col-tiling so the scores
    matmuls can run 2-way row-packed (K=64 only half-fills the PE array)
  - scores computed transposed [t, s]; exp on ScalarE straight out of PSUM
    with the 1/sqrt(D) scale folded into the activation's free scale field
    (scores are bounded ~+-3 so no max-subtraction is needed)
  - softmax denominator rides along as a 65th (ones) column of V
  - upd normalization via a partition-broadcast of the reciprocal; V bias is
    folded into b1 on the host (rows of probs sum to 1)
  - FFN biases ride the contraction as appended ones rows; final transpose
    back on the PE, contiguous DMA out.
"""

import sys

if "/opt/trn_rl_repo" not in sys.path:
    sys.path.insert(0, "/opt/trn_rl_repo")

import numpy as np
import ml_dtypes

import concourse.bass as bass
import concourse.tile as tile
from concourse import bacc, mybir
from concourse.bass import ds, ts
from concourse.masks import make_identity

# ---------------------------------------------------------------- constants
B, S, D, VOCAB, HID = 32, 2048, 64, 10000, 256
CORES = 8
NSEQ = B // CORES          # 4 sequences per core
P = 128
NT = S // P                # 16 t-blocks of 128
SCALE = 1.0 / float(np.sqrt(np.float32(64.0)))

F32 = mybir.dt.float32
BF16 = mybir.dt.bfloat16
FP8 = mybir.dt.float8e4
I16 = mybir.dt.int16
VAU_W = 80  # Vau row stride: DoubleRow needs the interleave step %16 == 0




_LDW_PATCHED = False


def _enable_walrus_ldw_opt():
    """Flip walrus's disabled-by-default LDWEIGHTS optimization pass."""
    global _LDW_PATCHED
    if _LDW_PATCHED:
        return
    from concourse import bass_utils as bu

    orig = bu.run_command

    def patched(cmd, **kw):
        cmd = ["--enable-ldw-opt=true" if c == "--enable-ldw-opt=false" else c
               for c in cmd]
        return orig(cmd, **kw)

    bu.run_command = patched
    _LDW_PATCHED = True


def build(n_seq: int = NSEQ):
    """Build + compile the per-core Bass program (identical on all cores)."""
    import os

    if os.environ.get("BASS_LDW_OPT"):
        _enable_walrus_ldw_opt()
    nc = bacc.Bacc("TRN2", target_bir_lowering=False, debug=False)

    xg_d = nc.dram_tensor("xg", [n_seq, P, S // 16], I16, kind="ExternalInput")
    embp_d = nc.dram_tensor("embp", [VOCAB, P], BF16, kind="ExternalInput")
    wqa_d = nc.dram_tensor("wqa", [D + 1, D], BF16, kind="ExternalInput")
    wka_d = nc.dram_tensor("wka", [D + 1, D], BF16, kind="ExternalInput")
    wv_d = nc.dram_tensor("wv", [D, D], BF16, kind="ExternalInput")
    w1a_d = nc.dram_tensor("w1a", [D + 1, HID], BF16, kind="ExternalInput")
    w2_d = nc.dram_tensor("w2", [HID, D], BF16, kind="ExternalInput")
    b2_d = nc.dram_tensor("b2", [D, 1], F32, kind="ExternalInput")
    out_d = nc.dram_tensor("out", [n_seq, S, D], F32, kind="ExternalOutput")

    with tile.TileContext(nc) as tc:
        _emit(nc, tc, n_seq, xg_d, embp_d, wqa_d, wka_d, wv_d, w1a_d, w2_d,
              b2_d, out_d)

    nc.compile()
    return nc


def _emit(nc, tc, n_seq, xg_d, embp_d, wqa_d, wka_d, wv_d, w1a_d, w2_d,
          b2_d, out_d):
    from contextlib import ExitStack

    ctx = ExitStack()
    const = ctx.enter_context(tc.tile_pool(name="const", bufs=1))
    sb = ctx.enter_context(tc.tile_pool(name="sb", bufs=2))
    # PSUM pools: scores 2x2 banks + upd 2 + misc 2 = 8 banks exactly.
    scp = ctx.enter_context(tc.tile_pool(name="scp", bufs=2, space="PSUM"))
    updp = ctx.enter_context(tc.tile_pool(name="updp", bufs=2, space="PSUM"))
    miscp = ctx.enter_context(tc.tile_pool(name="miscp", bufs=2, space="PSUM"))

    # ---- constants ----
    wqa_t = const.tile([D + 1, D], BF16)
    wka_t = const.tile([D + 1, D], BF16)
    wv_t = const.tile([D, D], BF16)
    w1a_t = const.tile([D + 1, HID], BF16)
    w2_t = const.tile([P, 2, D], BF16)
    b2_t = const.tile([D, 1], F32)
    ident_t = const.tile([D, D], F32)
    ones_t = const.tile([1, D + 1], BF16)
    nc.vector.memset(ones_t, 1.0)
    nc.sync.dma_start(out=wqa_t, in_=wqa_d.ap())
    nc.sync.dma_start(out=wka_t, in_=wka_d.ap())
    nc.sync.dma_start(out=wv_t, in_=wv_d.ap())
    nc.sync.dma_start(out=w1a_t, in_=w1a_d.ap())
    nc.sync.dma_start(out=w2_t, in_=w2_d.ap().rearrange("(h p) d -> p h d", p=P))
    nc.sync.dma_start(out=b2_t, in_=b2_d.ap())
    # HAM warmup: ~5us of dense dummy matmuls so the PE clock-gate opens
    # (K=8/8) before the real work; scattered sub-us gaps then keep it open.
    wup = miscp.tile([P, 512], F32, name="wup", tag="misc")
    for _ in range(24):
        nc.tensor.matmul(out=wup[0:D, 0:256], lhsT=wqa_t, rhs=w1a_t,
                         start=True, stop=True)
    # preload the exp activation table (~2.7us) off the critical path
    wux = const.tile([1, D], F32)
    nc.scalar.activation(out=wux, in_=wup[0:1, 0:D],
                         func=mybir.ActivationFunctionType.Exp, scale=0.001)

    # ---- per-seq state (python handles) ----
    idx = [None] * n_seq
    eTa = [None] * n_seq     # [128, S] bf16  (rows 0-63 e^T, row 64 ones)
    QT2 = [None] * n_seq     # [128, S] bf16  (rows 64-127 duplicate 0-63)
    KT2 = [None] * n_seq
    Vau = [None] * n_seq     # [128, NT, 65] bf16 (col 0 ones)
    expT = [None] * n_seq    # [128, NT, S] bf16
    updps = [dict() for _ in range(n_seq)]   # (j) -> psum tile [65, 512]
    updn = [dict() for _ in range(n_seq)]    # (j) -> sbuf [65, 512] bf16

    def proj_chunk(k, c):
        """Gather + Q^T/K^T (duplicated) + V_aug for 512-token chunk c of
        sequence k.  Chunk c provides QT2/KT2[:, 512c:512c+512] and V_aug
        t-blocks 4c..4c+3, so attention rounds can chase chunks."""
        if c == 0:
            idx[k] = sb.tile([P, S // 16], I16, name=f"idx{k}", tag="idx",
                             bufs=2)
            nc.sync.dma_start(out=idx[k], in_=xg_d.ap()[k])
            eTa[k] = sb.tile([P, S], BF16, name=f"eTa{k}", tag="eTa", bufs=2)
            QT2[k] = sb.tile([P, S], BF16, name=f"QT2{k}", tag="QT2", bufs=2)
            KT2[k] = sb.tile([P, S], BF16, name=f"KT2{k}", tag="KT2", bufs=2)
            # col 0 = ones: the softmax denominator rides as upd row 0.
            Vau[k] = sb.tile([P, NT, D + 1], BF16, name=f"Vau{k}", tag="Vau",
                             bufs=2)
            nc.vector.memset(Vau[k][:, :, 0:1], 1.0)
            expT[k] = sb.tile([P, NT, S], BF16, name=f"expT{k}", tag="expT",
                              bufs=2)
        CH = 512  # >512 idxs per gather overflows the SWDGE queue
        nc.gpsimd.dma_gather(
            out_ap=eTa[k][:, c * CH : (c + 1) * CH].unsqueeze(1),
            in_ap=embp_d.ap(),
            idxs_ap=idx[k][:, c * (CH // 16) : (c + 1) * (CH // 16)],
            num_idxs=CH,
            num_idxs_reg=CH,
            elem_size=P,
            transpose=True,
        )
        for w_t, dst in ((wqa_t, QT2[k]), (wka_t, KT2[k])):
            pps = miscp.tile([P, 512], F32, name=f"pps{k}_{c}", tag="misc")
            rhs = eTa[k][0 : D + 1, ts(c, 512)]
            nc.tensor.matmul(out=pps[0:D, :], lhsT=w_t, rhs=rhs,
                             start=True, stop=True, tile_position=(0, 0))
            nc.tensor.matmul(out=pps[D:P, :], lhsT=w_t, rhs=rhs,
                             start=True, stop=True, tile_position=(0, 64))
            nc.vector.tensor_copy(out=dst[:, ts(c, 512)], in_=pps)
        for i in range(4 * c, 4 * c + 4):
            vps = miscp.tile([P, D], F32, name=f"vps{k}_{i}", tag="misc")
            nc.tensor.matmul(out=vps, lhsT=eTa[k][0:D, ts(i, P)], rhs=wv_t,
                             start=True, stop=True)
            nc.vector.tensor_copy(out=Vau[k][:, i, 1 : D + 1], in_=vps)

    def att_round(k, p, sq):
        """Scores for t-blocks (2p, 2p+1) x s-quarter sq + exp."""
        t0, t1 = 2 * p, 2 * p + 1
        sc = scp.tile([P, 1024], F32, name=f"sc{k}_{sq}_{p}", tag="sc")
        nc.tensor.matmul(out=sc[:, 0:512],
                         lhsT=KT2[k][0:D, ts(t0, P)],
                         rhs=QT2[k][0:D, ts(sq, 512)],
                         start=True, stop=True, tile_position=(0, 0))
        nc.tensor.matmul(out=sc[:, 512:1024],
                         lhsT=KT2[k][D:P, ts(t1, P)],
                         rhs=QT2[k][D:P, ts(sq, 512)],
                         start=True, stop=True, tile_position=(64, 0))
        nc.scalar.activation(
            out=expT[k][:, t0 : t0 + 2, ts(sq, 512)],
            in_=sc.rearrange("x (a b) -> x a b", a=2),
            func=mybir.ActivationFunctionType.Exp,
            scale=SCALE,
        )

    def upd_round(k, p, sq):
        """upd accumulation MMs for a round whose exp finished ~2 rounds ago
        (the lag keeps these off the scores->exp dependency chain)."""
        j = sq
        if j not in updps[k]:
            updps[k][j] = updp.tile([D + 1, 512], F32,
                                    name=f"upd{k}_{j}", tag="upd")
        for t in (2 * p, 2 * p + 1):
            nc.tensor.matmul(out=updps[k][j],
                             lhsT=Vau[k][:, t, 0 : D + 1],
                             rhs=expT[k][:, t, ts(j, 512)],
                             start=(t == 0), stop=(t == NT - 1))
        if p == 7:
            normalize(k, sq)

    def normalize(k, j):
        """updn = upd / denom (denom = row 0); row 0 becomes ~1.0.

        The reciprocal is computed on the [1, 512] denominator row, then
        replicated across partitions with a K=1 fp32 ones-matmul on the PE
        (DVE lanes cannot cross partitions and gpsimd broadcast is slow)."""
        ups = updps[k].pop(j)
        updu = sb.tile([D + 1, 512], F32, name=f"updu{k}_{j}", tag="updu",
                       bufs=2)
        nc.vector.tensor_copy(out=updu, in_=ups)
        recd = sb.tile([1, 512], F32, name=f"recd{k}_{j}", tag="recd", bufs=2)
        nc.vector.reciprocal_approx_fast(out=recd, in_=updu[0:1, :])
        recb = sb.tile([1, 512], BF16, name=f"recb{k}_{j}", tag="recb", bufs=2)
        nc.vector.tensor_copy(out=recb, in_=recd)
        rec_ps = miscp.tile([D + 1, 512], F32, name=f"recps{k}_{j}",
                            tag="misc")
        nc.tensor.matmul(out=rec_ps, lhsT=ones_t, rhs=recb,
                         start=True, stop=True)
        updn[k][j] = sb.tile([D + 1, 512], BF16, name=f"updn{k}_{j}",
                             tag="updn", bufs=8)
        nc.vector.tensor_mul(out=updn[k][j], in0=updu, in1=rec_ps)

    def ffn(k, j):
        """relu FFN + out-proj + transpose + store for s-block j of seq k."""
        un = updn[k].pop(j)
        hn = []
        for half in range(2):
            hps = miscp.tile([P, 512], F32, name=f"hps{k}_{j}_{half}",
                             tag="misc")
            nc.tensor.matmul(out=hps, lhsT=w1a_t[:, ts(half, P)], rhs=un,
                             start=True, stop=True)
            hnt = sb.tile([P, 512], BF16, name=f"hn{k}_{j}_{half}", tag="hn",
                          bufs=4)
            nc.vector.tensor_scalar(out=hnt, in0=hps, scalar1=0.0,
                                    scalar2=None, op0=mybir.AluOpType.max)
            hn.append(hnt)
        ops = miscp.tile([D, 512], F32, name=f"ops{k}_{j}", tag="misc")
        for half in range(2):
            nc.tensor.matmul(out=ops, lhsT=w2_t[:, half, :], rhs=hn[half],
                             start=(half == 0), stop=(half == 1))
        outT = sb.tile([D, 512], F32, name=f"outT{k}_{j}", tag="outT", bufs=2)
        nc.vector.tensor_scalar(out=outT, in0=ops, scalar1=b2_t,
                                scalar2=None, op0=mybir.AluOpType.add)
        ob = sb.tile([P, 4, D], F32, name=f"ob{k}_{j}", tag="ob", bufs=2)
        for q in range(4):
            tps = miscp.tile([P, D], F32, name=f"tps{k}_{j}_{q}", tag="misc")
            nc.tensor.transpose(out=tps, in_=outT[:, ts(q, P)],
                                identity=ident_t)
            nc.vector.tensor_copy(out=ob[:, q, :], in_=tps)
        dst = out_d.ap()[k][ts(j, 512)].rearrange("(q p) d -> p q d", p=P)
        nc.sync.dma_start(out=dst, in_=ob)

    # ------------------------------- schedule -------------------------------
    # s-quarter outer, t-pair inner.  upd MMs trail their round by UPD_LAG
    # rounds so the scores->exp chain never waits on them; normalize rides
    # the lagged upd close.  seq 0's projection chunks are emitted just ahead
    # of the rounds that first need them; seq k+1's chunks are prefetched
    # inside att(k)'s sq=1 rounds.  FFN for quarter j runs interleaved into
    # quarter j+1's rounds; ffn(k, 3) lands in att(k+1)'s rounds.
    from collections import deque

    UPD_LAG = 2
    pending = deque()
    for k in range(n_seq):
        for sq in range(4):
            for p in range(8):
                if k == 0 and sq == 0 and p % 2 == 0:
                    proj_chunk(0, p // 2)
                if k == 0 and sq == 0 and p == 1:
                    # identity for the output transposes -- emitted here so
                    # its gpsimd ops don't delay seq 0's gathers on the queue
                    make_identity(nc, ident_t)
                att_round(k, p, sq)
                pending.append((k, p, sq))
                if len(pending) > UPD_LAG:
                    upd_round(*pending.popleft())
                if p == 2:
                    if sq == 0 and k > 0:
                        ffn(k - 1, 3)
                    elif sq >= 1:
                        ffn(k, sq - 1)
                if sq == 1 and p % 2 == 1 and k + 1 < n_seq:
                    proj_chunk(k + 1, p // 2)
    while pending:
        upd_round(*pending.popleft())
    ffn(n_seq - 1, 3)
    ctx.close()


# ---------------------------------------------------------------- host side

def _prep_params(inputs):
    """Host-side parameter prep (layout changes + bias folds only)."""
    f = {k: np.asarray(v) for k, v in inputs.items()}
    emb = f["emb"].astype(np.float32)
    embp = np.zeros((VOCAB, P), dtype=ml_dtypes.bfloat16)
    embp[:, :D] = emb.astype(ml_dtypes.bfloat16)
    embp[:, D] = 1.0
    wqa = np.concatenate([f["wq"], f["bq"][None, :]], 0).astype(ml_dtypes.bfloat16)
    wka = np.concatenate([f["wk"], f["bk"][None, :]], 0).astype(ml_dtypes.bfloat16)
    b1p = (f["b1"].astype(np.float64)
           + f["bv"].astype(np.float64) @ f["w1"].astype(np.float64))
    # bias row FIRST: updn row 0 is the (denom * 1/denom) ~= 1.0 ones row
    w1a = np.concatenate([b1p[None, :].astype(np.float32), f["w1"]], 0).astype(
        ml_dtypes.bfloat16)
    return {
        "embp": embp,
        "wqa": wqa,
        "wka": wka,
        "wv": f["wv"].astype(ml_dtypes.bfloat16),
        "w1a": w1a,
        "w2": f["w2"].astype(ml_dtypes.bfloat16),
        "b2": f["b2"].astype(np.float32).reshape(D, 1),
    }


def _prep_xg(x_shard):
    """Pack token ids into dma_gather's index layout: [seq, 128, S//16] i16,
    idx[p, f] = x[f*16 + p%16], replicated across the 8 Q7 core stripes."""
    n_seq = x_shard.shape[0]
    xg = np.empty((n_seq, P, S // 16), dtype=np.int16)
    for s_i in range(n_seq):
        m = x_shard[s_i].astype(np.int16).reshape(S // 16, 16).T  # [16, S/16]
        xg[s_i] = np.tile(m, (8, 1))
    return xg


_CACHED_NC = None
LAST_EXEC_NS = None


def _install_ntff_hook():
    """Expose the axon NTFF profiling hook that bass_utils looks for."""
    import types

    if "antenv.axon_hooks" in sys.modules:
        return
    try:
        from trn_agent_boot.trn_boot import _ntff_profile_via_ctypes

        hook = _ntff_profile_via_ctypes("/opt/axon/libaxon_pjrt.so")
    except Exception:
        return
    m = types.ModuleType("antenv.axon_hooks")
    m.get_axon_ntff_profile_hook = lambda: hook
    m.set_axon_ntff_profile_hook = lambda h: None
    sys.modules["antenv.axon_hooks"] = m


def kernel(**inputs) -> np.ndarray:
    global _CACHED_NC, LAST_EXEC_NS
    import os
    from concourse import bass_utils

    params = _prep_params(inputs)
    x = np.asarray(inputs["x"]).astype(np.int64)
    assert x.shape == (B, S)

    if _CACHED_NC is None:
        _CACHED_NC = build(NSEQ)
    nc = _CACHED_NC

    in_maps = []
    for c in range(CORES):
        shard = x[c * NSEQ : (c + 1) * NSEQ]
        m = dict(params)
        m["xg"] = _prep_xg(shard)
        in_maps.append(m)

    trace = bool(os.environ.get("BASS_KERNEL_TRACE"))
    kw = {}
    if trace:
        _install_ntff_hook()
        kw = {"trace": True,
              "tmpdir": os.environ.get("BASS_KERNEL_TRACE_DIR") or None}
    res = bass_utils.run_bass_kernel_spmd(nc, in_maps,
                                          core_ids=list(range(CORES)), **kw)
    LAST_EXEC_NS = res.exec_time_ns
    out = np.concatenate([r["out"] for r in res.results], axis=0)
    return out.astype(np.float32)
